# revision 52
# baseline (speedup 1.0000x reference)
"""BiLSTM-over-word2vec Trainium2 kernel (8 NeuronCores, SPMD).

Strategy
--------
Data-parallel over the token axis: core c owns tokens [c*1024, (c+1)*1024).
The inherently-sequential LSTM scan is parallelized with chunked warmup:
the LSTM forgets exponentially, so a chunk of L tokens warmed up from zero
state over W extra leading steps reproduces the exact scan state to ~1e-6
by the time real outputs start. Each core runs B = 1024/L chunks per
direction as a batch, so the scan is W+L sequential *batched* steps.

On-chip layout: gates-on-partitions, hidden padded 200->256, gate order
[i, f, o, g~] with g~ pre-scaled x2 so ONE sigmoid covers all gates
(tanh(x) = 2*sigmoid(2x)-1).

Token storage is (j, c)-ordered: the gathered tokens are permuted on the
host so that within each 128-token group, partition p = 8*j + c_local
(j = within-chunk offset, c = chunk). eT/exT are [.., 16 j, 66 c] grids;
every scan step's ex slice is then a CONTIGUOUS 64-column run (v2's
token-ordered exT made the per-step N=512 inject matmul read stride-32B
and cost 4x).

h state lives in contiguous [128, L, 2, B] per-direction buffers indexed
by within-chunk offset, so the recurrent matmul rhs is contiguous.

The embedding table is host-prepped: relu'd, bf16, padded to 384 cols
with a constant bias column (=1) and a flag row at index VOCAB for
out-of-range warmup tokens (-30 on i,f gate columns of Wih freezes
state).
"""

import os
import sys

for _p in ("/opt/trn_rl_repo", "/root/.axon_site/_ro/trn_rl_repo"):
    if os.path.isdir(_p) and _p not in sys.path:
        sys.path.insert(0, _p)

import numpy as np
import ml_dtypes

import concourse.bass as bass
import concourse.mybir as mybir
import concourse.tile as tile
from concourse import bacc
from concourse.bass import IndirectOffsetOnAxis
from concourse.masks import make_identity

BF16 = ml_dtypes.bfloat16

# problem constants (hardcoded per contract)
VOCAB, E, H, EXTRA, OUT, T = 100000, 300, 200, 50, 2, 8192
HP = 256          # padded hidden
G = 4 * HP        # 1024 padded gate rows
NC = 8
SPAN = T // NC    # 1024 tokens per core
L = 16            # chunk length
W = 9             # warmup steps
B = SPAN // L     # 64 chunks per direction per core
STEPS = L + W     # 28
GW = 16           # gather halo (16-aligned so the (j,c) grid is clean)
COLS = SPAN + 2 * GW         # 1056 gathered token slots per core
CPAD = 1152                  # padded to 9 gather groups of 128
NGT = CPAD // 128
CG = COLS // L + 2           # 66 c-columns in the (j,c) grid
EA = 384          # augmented embedding width: 300 emb + bias + flag + pad
F32 = mybir.dt.float32
BF = mybir.dt.bfloat16
SIG = mybir.ActivationFunctionType.Sigmoid
TANH = mybir.ActivationFunctionType.Tanh
RELU = mybir.ActivationFunctionType.Relu
MULT = mybir.AluOpType.mult
ADD = mybir.AluOpType.add
SUB = mybir.AluOpType.subtract

_GATE_SRC = (0, 200, 600, 400)  # gate order [i, f, o, g~] -> orig i,f,g,o offsets


def _reorder_rows(M4h, scale_g=2.0):
    """[4H(orig i,f,g,o), ...] -> [G rows in order i,f,o,g~], g~ scaled."""
    out = np.zeros((G,) + M4h.shape[1:], np.float32)
    for gi, src in enumerate(_GATE_SRC):
        blk = M4h[src:src + H].astype(np.float32)
        if gi == 3:
            blk = blk * scale_g
        out[gi * HP: gi * HP + H] = blk
    return out


def _bf16_hi_lo(a):
    hi = a.astype(BF16)
    lo = (a.astype(np.float32) - hi.astype(np.float32)).astype(BF16)
    return hi, lo


def _prep_weights(Wih_f, Whh_f, b_f, Wih_b, Whh_b, b_b, W_h2s, b_h2s, W_s2o, b_s2o):
    """Host-side weight reordering/padding; returns dict of DRAM input arrays
    shared by all cores (everything except the embedding table and indices)."""
    whh = np.zeros((128, 2, 8, 2, 128), BF16)
    wih = np.zeros((128, 2, 3, G), BF16)
    for d, (Wih_d, Whh_d, b_d) in enumerate(
        ((Wih_f, Whh_f, b_f), (Wih_b, Whh_b, b_b))
    ):
        Whh_r = np.zeros((G, HP), np.float32)
        Whh_r[:, :H] = _reorder_rows(Whh_d)
        whh_bf = Whh_r.astype(BF16)
        for m in range(8):
            for k in range(2):
                # lhsT tile [K=128 (h dims), M=128 (gate rows)]
                whh[:, d, m, k, :] = whh_bf[m * 128:(m + 1) * 128,
                                            k * 128:(k + 1) * 128].T
        Wih_aug = np.zeros((EA, G), np.float32)
        Wih_aug[:E, :] = _reorder_rows(Wih_d).T          # [300, G]
        Wih_aug[E, :] = _reorder_rows(b_d[:, None])[:, 0]  # bias row (col 300=1)
        flagrow = np.zeros(G, np.float32)
        flagrow[:512] = -30.0                             # i,f gate columns
        Wih_aug[E + 1, :] = flagrow                       # flag row (col 301)
        wih[:, d, :, :] = np.stack(
            [Wih_aug[k * 128:(k + 1) * 128].astype(BF16) for k in range(3)], axis=1
        )
    # MLP weights: K space = [hf(256 pad) ; hb(256 pad)] = 512 rows
    W1p = np.zeros((512, 64), np.float32)
    W1p[0:H, :EXTRA] = W_h2s.T[0:H]
    W1p[256:256 + H, :EXTRA] = W_h2s.T[H:2 * H]
    w1hi, w1lo = _bf16_hi_lo(W1p)
    w2s = np.zeros((128, 4, 2, 64), BF16)
    for k in range(4):
        w2s[:, k, 0, :] = w1hi[k * 128:(k + 1) * 128]
        w2s[:, k, 1, :] = w1lo[k * 128:(k + 1) * 128]
    W2p = np.zeros((64, OUT), np.float32)
    W2p[:EXTRA] = W_s2o.T
    w2hi, w2lo = _bf16_hi_lo(W2p)
    ws2o = np.zeros((64, 2, OUT), BF16)
    ws2o[:, 0, :] = w2hi
    ws2o[:, 1, :] = w2lo
    b1 = np.zeros((64, 1), np.float32)
    b1[:EXTRA, 0] = b_h2s.astype(np.float32)
    b2b = np.tile(np.asarray(b_s2o, np.float32).reshape(1, 1, OUT), (128, 4, 1))
    return dict(whh_w=whh, wih_w=wih, w2s_w=w2s, ws2o_w=ws2o, b1=b1, b2b=b2b,
                ident=np.eye(128, dtype=BF16))


def _prep_emb(emb):
    """relu'd bf16 embedding, padded to EA cols with bias col and flag row."""
    ea = np.zeros((VOCAB + 1, EA), BF16)
    ea[:VOCAB, :E] = np.maximum(np.asarray(emb, np.float32), 0.0)
    ea[:, E] = 1.0          # bias column: every gathered token contributes b
    ea[VOCAB, E + 1] = 1.0  # flag column set only on the invalid-token row
    return ea


def _prep_core_inputs(x, core):
    """Per-core token indices [128, NGT], (j,c)-permuted within each group:
    gathered slot s = 128*g + 16*c_local + j lands on partition 8*j + c_local,
    so the PE transpose emits (j, c)-ordered columns. Invalid slots -> VOCAB."""
    base = core * SPAN
    toks = np.arange(base - GW, base + SPAN + GW, dtype=np.int64)
    invalid = (toks < 0) | (toks >= T)
    tokc = np.clip(toks, 0, T - 1)
    xi = x[tokc].astype(np.int64)
    xi = np.where(xi < 0, 0, xi)          # masked tokens never occur (randint)
    xi = np.where(invalid, VOCAB, xi)
    idx = np.zeros(CPAD, np.int32)
    idx[:COLS] = xi.astype(np.int32)
    idxg = idx.reshape(NGT, 8, L)          # [g, c_local, j]
    idxp = np.ascontiguousarray(idxg.transpose(2, 1, 0)).reshape(128, NGT)
    return dict(xidx=idxp.copy())


def make_in_maps(x, emb, Wih_f, Whh_f, b_f, Wih_b, Whh_b, b_b,
                 W_h2s, b_h2s, W_s2o, b_s2o):
    shared = _prep_weights(Wih_f, Whh_f, b_f, Wih_b, Whh_b, b_b,
                           W_h2s, b_h2s, W_s2o, b_s2o)
    shared["emb"] = _prep_emb(emb)
    x = np.asarray(x)
    in_maps = []
    for core in range(NC):
        m = dict(shared)
        m.update(_prep_core_inputs(x, core))
        in_maps.append(m)
    return in_maps


def build_nc():
    nc = bacc.Bacc("TRN2", target_bir_lowering=False, debug=False, num_devices=NC)

    emb_t = nc.dram_tensor("emb", [VOCAB + 1, EA], BF, kind="ExternalInput").ap()
    xidx_t = nc.dram_tensor("xidx", [128, NGT], mybir.dt.int32, kind="ExternalInput").ap()
    ident_t = nc.dram_tensor("ident", [128, 128], BF, kind="ExternalInput").ap()
    whh_t = nc.dram_tensor("whh_w", [128, 2, 8, 2, 128], BF, kind="ExternalInput").ap()
    wih_t = nc.dram_tensor("wih_w", [128, 2, 3, G], BF, kind="ExternalInput").ap()
    w2s_t = nc.dram_tensor("w2s_w", [128, 4, 2, 64], BF, kind="ExternalInput").ap()
    ws2o_t = nc.dram_tensor("ws2o_w", [64, 2, OUT], BF, kind="ExternalInput").ap()
    b1_t = nc.dram_tensor("b1", [64, 1], F32, kind="ExternalInput").ap()
    b2b_t = nc.dram_tensor("b2b", [128, 4, OUT], F32, kind="ExternalInput").ap()
    out_t = nc.dram_tensor("out", [SPAN, OUT], F32, kind="ExternalOutput").ap()

    with tile.TileContext(nc) as tc:
        with tc.tile_pool(name="const", bufs=1) as const:
            # DMA priority: idx+ident gate the gathers/spin, wih gates the exT
            # matmuls (and must beat the gather descriptor flood); whh is not
            # needed until scan step 1, so it goes last.
            idx_sb = const.tile([128, NGT], mybir.dt.int32, tag="idx")
            nc.sync.dma_start(out=idx_sb[:], in_=xidx_t)
            identb = const.tile([128, 128], BF, tag="identb")
            nc.sync.dma_start(out=identb[:], in_=ident_t)
            wih_sb = const.tile([128, 2, 3, G], BF, tag="wih")
            nc.sync.dma_start(out=wih_sb[:], in_=wih_t)
            w2s_sb = const.tile([128, 4, 2, 64], BF, tag="w2s")
            nc.sync.dma_start(out=w2s_sb[:], in_=w2s_t)
            ws2o_sb = const.tile([64, 2, OUT], BF, tag="ws2o")
            nc.sync.dma_start(out=ws2o_sb[:], in_=ws2o_t)
            b1_sb = const.tile([64, 1], F32, tag="b1")
            nc.sync.dma_start(out=b1_sb[:], in_=b1_t)
            b2b_sb = const.tile([128, 4, OUT], F32, tag="b2b")
            nc.sync.dma_start(out=b2b_sb[:], in_=b2b_t)
            whh_sb = const.tile([128, 2, 8, 2, 128], BF, tag="whh")
            nc.sync.dma_start(out=whh_sb[:], in_=whh_t)
            ident = identb

            # (j, c) grids
            eT = [const.tile([128, L, CG], BF, tag=f"eT{k}", name=f"eT{k}")
                  for k in range(3)]
            exT = [const.tile([128, 8, L, CG], BF, tag=f"exT{d}", name=f"exT{d}")
                   for d in range(2)]
            # h state: [128 hdim-parts, L within-chunk slot, 2 hdim-halves, B]
            hbuf = [const.tile([128, L, 2, B], BF, tag=f"hb{d}", name=f"hb{d}")
                    for d in range(2)]
            # staged MLP outputs: rows 0:64 slot s = token (c, j=s);
            # rows 64:128 slot s = token (c, j=8+s)
            obuf = const.tile([128, 8, OUT], F32, tag="obuf")

            # scan state pools (outer: step 0 is emitted inside the gather
            # block so it isn't FIFO-blocked behind direction 1's exT copies)
            scan_stack = tc.tile_pool(name="act", bufs=3)
            ap_ = scan_stack.__enter__()
            cstate_stack = tc.tile_pool(name="cstate", bufs=3)
            cp = cstate_stack.__enter__()
            scr_stack = tc.tile_pool(name="scr", bufs=3)
            scr = scr_stack.__enter__()

            def ex_slice(d, sp):
                q = (sp + GW - W) if d == 0 else (GW + L + W - 1 - sp)
                jj, cl = q % L, q // L
                return exT[d][:, :, jj, cl:cl + B]

            c_prev = [None, None]
            h_prev = [None, None]

            def hdst_for(d, sp):
                j = (sp - W) if d == 0 else (L + W - 1 - sp)
                if sp >= W:
                    return hbuf[d][:, j, :, :]
                hw = scr.tile([128, 2, B], BF, tag=f"hw{d}", name=f"hw{d}")
                return hw[:]

            def sp0_dir(d):
                # step 0, one direction: no h yet -> gates = sigmoid(ex)
                a0 = ap_.tile([128, 8, B], F32, tag=f"a{d}", name=f"a{d}")
                nc.scalar.activation(a0[:], ex_slice(d, 0), SIG)
                t0 = scr.tile([128, 2, B], F32, tag=f"t{d}", name=f"t{d}")
                cn0 = cp.tile([128, 2, B], F32, tag=f"c{d}", name=f"c{d}")
                tc0 = scr.tile([128, 2, B], F32, tag=f"tc{d}", name=f"tc{d}")
                nc.vector.tensor_tensor(
                    out=t0[:], in0=a0[:, 0:2, :], in1=a0[:, 6:8, :], op=MULT)
                nc.vector.scalar_tensor_tensor(
                    out=cn0[:], in0=t0[:], scalar=2.0,
                    in1=a0[:, 0:2, :], op0=MULT, op1=SUB)
                nc.scalar.activation(tc0[:], cn0[:], TANH)
                hd = hdst_for(d, 0)
                nc.gpsimd.tensor_tensor(
                    out=hd, in0=a0[:, 4:6, :], in1=tc0[:], op=MULT)
                h_prev[d] = hd
                c_prev[d] = cn0[:]

            # ---- gather + transpose + exT, interleaved so exT matmuls start
            # after only the gather groups they need (Tensor queue is FIFO) ----
            cslabs = [(0, 26), (26, 26), (52, 14)]
            with (
                tc.tile_pool(name="gath", bufs=1) as gp,
                tc.tile_pool(name="gpsum", bufs=4, space="PSUM") as gps,
                tc.tile_pool(name="expsum", bufs=3, space="PSUM") as exps,
            ):
                # gathers first: nothing on gpsimd before them, so the
                # indirect-DMA issues (~1.4us each, serial) start immediately
                ets = []
                for g in range(NGT):
                    et = gp.tile([128, EA], BF, tag=f"ge{g}", name=f"ge{g}")
                    nc.gpsimd.indirect_dma_start(
                        out=et[:],
                        out_offset=None,
                        in_=emb_t,
                        in_offset=IndirectOffsetOnAxis(ap=idx_sb[:, g:g + 1], axis=0),
                    )
                    ets.append(et)

                # PE spin: lift the HAM clock gate before the transposes
                with tc.tile_pool(name="warm", bufs=1, space="PSUM") as wp:
                    wps = wp.tile([128, 128], F32, tag="warm")
                    for _ in range(16):
                        nc.tensor.matmul(out=wps[:], lhsT=ident[:], rhs=ident[:],
                                         start=True, stop=True)

                def emit_transposes(glist):
                    for g in glist:
                        cw = 8 if g < NGT - 1 else 2   # last group: 2 real c's
                        for kc in range(3):
                            pt = gps.tile([128, L, 8], BF, tag="tr", name="pt")
                            nc.tensor.transpose(
                                out=pt[:], in_=ets[g][:, kc * 128:(kc + 1) * 128],
                                identity=ident[:],
                            )
                            if (g + kc) % 2 == 0:
                                nc.vector.tensor_copy(
                                    out=eT[kc][:, :, 8 * g:8 * g + cw],
                                    in_=pt[:, :, :cw],
                                )
                            else:
                                nc.scalar.copy(
                                    out=eT[kc][:, :, 8 * g:8 * g + cw],
                                    in_=pt[:, :, :cw],
                                )

                def emit_exslab(d, si):
                    c0, cn = cslabs[si]
                    for m in range(8):
                        ps = exps.tile([128, L, 26], F32, tag="exps",
                                       name="exps")
                        if cn == 26:
                            mm_out = cp_in = ps[:]
                        else:  # pack short slab contiguously in PSUM
                            flat = ps[:].rearrange("p j c -> p (j c)")
                            mm_out = flat[:, :L * cn]
                            cp_in = mm_out.rearrange("p (j c) -> p j c",
                                                     j=L, c=cn)
                        for k in range(3):
                            nc.tensor.matmul(
                                out=mm_out,
                                lhsT=wih_sb[:, d, k, m * 128:(m + 1) * 128],
                                rhs=eT[k][:, :, c0:c0 + cn],
                                start=(k == 0),
                                stop=(k == 2),
                            )
                        if (d + m + si) % 2 == 0:
                            nc.vector.tensor_copy(
                                out=exT[d][:, m, :, c0:c0 + cn], in_=cp_in)
                        else:
                            nc.scalar.copy(
                                out=exT[d][:, m, :, c0:c0 + cn], in_=cp_in)

                emit_transposes([0, 1, 2, 3])
                emit_exslab(0, 0)
                emit_transposes([4, 5, 6])
                emit_exslab(1, 0)
                emit_transposes([7, 8])
                emit_exslab(0, 1)
                emit_exslab(1, 1)
                emit_exslab(0, 2)
                sp0_dir(0)          # d0 step 0 runs while d1's last slab computes
                emit_exslab(1, 2)
                sp0_dir(1)

            # ---- the scan, steps 1.., with the MLP head fused in as
            # per-j-pair passes (pair p = offsets (7-p, 8+p) is complete
            # after step 20+p; its MLP matmuls replace dummy drip) ----
            out_j = out_t.rearrange("(c j) o -> j c o", j=L)
            with (
                tc.tile_pool(name="pg", bufs=2, space="PSUM") as pgp,
                tc.tile_pool(name="dummy", bufs=1, space="PSUM") as dpp,
                tc.tile_pool(name="mlp", bufs=1, space="PSUM") as mpp,
            ):
                dps = dpp.tile([128, 128], F32, tag="dummy")

                def emit_mlp_pair(p):
                    jlo, jhi = 7 - p, 8 + p
                    ps = mpp.tile([64, 128], F32, tag="mps", name="mps")
                    mmi = 0
                    for d in range(2):
                        for k in range(2):
                            for hl in range(2):
                                nc.tensor.matmul(
                                    out=ps[:],
                                    lhsT=w2s_sb[:, d * 2 + k, hl, :],
                                    rhs=hbuf[d][:, jlo:jhi + 1:(jhi - jlo), k, :],
                                    start=(mmi == 0), stop=(mmi == 7),
                                )
                                mmi += 1
                    s32 = scr.tile([64, 128], F32, tag="ms32", name="ms32")
                    nc.scalar.activation(s32[:], ps[:], RELU, bias=b1_sb[:])
                    shi = scr.tile([64, 128], BF, tag="mshi", name="mshi")
                    nc.vector.tensor_copy(out=shi[:], in_=s32[:])
                    slo = scr.tile([64, 128], BF, tag="mslo", name="mslo")
                    nc.vector.tensor_tensor(
                        out=slo[:], in0=s32[:], in1=shi[:], op=SUB)
                    po = mpp.tile([128, OUT], F32, tag="mpo", name="mpo")
                    for oi, (shl, whl) in enumerate(((shi, 0), (shi, 1), (slo, 0))):
                        nc.tensor.matmul(
                            out=po[:], lhsT=shl[:], rhs=ws2o_sb[:, whl, :],
                            start=(oi == 0), stop=(oi == 2),
                        )
                    nc.vector.tensor_tensor(
                        out=obuf[0:64, jlo, :], in0=po[0:64, :],
                        in1=b2b_sb[0:64, 0, :], op=ADD)
                    nc.vector.tensor_tensor(
                        out=obuf[64:128, jhi - 8, :], in0=po[64:128, :],
                        in1=b2b_sb[64:128, 0, :], op=ADD)

                for sp in range(1, STEPS):
                    a = [ap_.tile([128, 8, B], F32, tag=f"a{d}", name=f"a{d}")
                         for d in range(2)]
                    pss = [pgp.tile([128, 8, B], F32, tag=f"pg{d}", name=f"pg{d}")
                           for d in range(2)]
                    # ex inject: one contiguous N=512 identity matmul per dir
                    for d in range(2):
                        nc.tensor.matmul(
                            out=pss[d][:], lhsT=ident[:], rhs=ex_slice(d, sp),
                            start=True, stop=False,
                        )
                    # dummy drip right before the h-dependent matmuls: keeps
                    # the PE non-idle while the previous step's chain finishes
                    # (the HAM clock gate re-throttles on any idle window);
                    # late steps get real MLP matmuls instead
                    for _ in range(38 if sp < W + 9 else 16):
                        nc.tensor.matmul(out=dps[:, :64], lhsT=ident[:],
                                         rhs=ident[:, :64], start=True, stop=True)
                    for m in range(8):
                        for k in range(2):
                            nc.tensor.matmul(
                                out=pss[0][:, m, :],
                                lhsT=whh_sb[:, 0, m, k, :],
                                rhs=h_prev[0][:, k, :],
                                start=False, stop=(k == 1),
                            )
                    nc.scalar.activation(a[0][:], pss[0][:], SIG)
                    for m in range(8):
                        for k in range(2):
                            nc.tensor.matmul(
                                out=pss[1][:, m, :],
                                lhsT=whh_sb[:, 1, m, k, :],
                                rhs=h_prev[1][:, k, :],
                                start=False, stop=(k == 1),
                            )
                    if sp >= W + 9:
                        emit_mlp_pair(sp - (W + 9))
                    # gate math; engine queues ordered by operand readiness:
                    #   ACT: [sig0, sig1, tanh0, tanh1]
                    #   DVE: [t0, u0, c0, h0, t1, u1, c1, h1]
                    #   GpSimd: [r0, r1]
                    t = [scr.tile([128, 2, B], F32, tag=f"t{d}", name=f"t{d}")
                         for d in range(2)]
                    u = [scr.tile([128, 2, B], F32, tag=f"u{d}", name=f"u{d}")
                         for d in range(2)]
                    r = [scr.tile([128, 2, B], F32, tag=f"r{d}", name=f"r{d}")
                         for d in range(2)]
                    cnew = [cp.tile([128, 2, B], F32, tag=f"c{d}", name=f"c{d}")
                            for d in range(2)]
                    tct = [scr.tile([128, 2, B], F32, tag=f"tc{d}", name=f"tc{d}")
                          for d in range(2)]
                    hd = [None, None]

                    nc.vector.tensor_tensor(
                        out=t[0][:], in0=a[0][:, 0:2, :], in1=a[0][:, 6:8, :], op=MULT)
                    nc.vector.scalar_tensor_tensor(
                        out=u[0][:], in0=t[0][:], scalar=2.0,
                        in1=a[0][:, 0:2, :], op0=MULT, op1=SUB)
                    nc.scalar.activation(a[1][:], pss[1][:], SIG)
                    nc.gpsimd.tensor_tensor(
                        out=r[0][:], in0=a[0][:, 2:4, :], in1=c_prev[0], op=MULT)
                    nc.vector.tensor_tensor(
                        out=cnew[0][:], in0=r[0][:], in1=u[0][:], op=ADD)
                    nc.scalar.activation(tct[0][:], cnew[0][:], TANH)
                    hd[0] = hdst_for(0, sp)
                    nc.vector.tensor_tensor(
                        out=hd[0], in0=a[0][:, 4:6, :], in1=tct[0][:], op=MULT)
                    nc.gpsimd.tensor_tensor(
                        out=r[1][:], in0=a[1][:, 2:4, :], in1=c_prev[1], op=MULT)
                    nc.vector.tensor_tensor(
                        out=t[1][:], in0=a[1][:, 0:2, :], in1=a[1][:, 6:8, :], op=MULT)
                    nc.vector.scalar_tensor_tensor(
                        out=u[1][:], in0=t[1][:], scalar=2.0,
                        in1=a[1][:, 0:2, :], op0=MULT, op1=SUB)
                    nc.vector.tensor_tensor(
                        out=cnew[1][:], in0=r[1][:], in1=u[1][:], op=ADD)
                    nc.scalar.activation(tct[1][:], cnew[1][:], TANH)
                    hd[1] = hdst_for(1, sp)
                    nc.vector.tensor_tensor(
                        out=hd[1], in0=a[1][:, 4:6, :], in1=tct[1][:], op=MULT)
                    for d in range(2):
                        h_prev[d] = hd[d]
                        c_prev[d] = cnew[d][:]

                emit_mlp_pair(7)   # last pair (j = 0, 15)
                # two contiguous output DMAs (row t = 16c + j2*8 + jj)
                out_r2 = out_t.rearrange("(c j2 jj) o -> j2 c jj o",
                                         j2=2, jj=8)
                nc.sync.dma_start(out=out_r2[0], in_=obuf[0:64, :, :])
                nc.sync.dma_start(out=out_r2[1], in_=obuf[64:128, :, :])

            scr_stack.__exit__(None, None, None)
            cstate_stack.__exit__(None, None, None)
            scan_stack.__exit__(None, None, None)

    nc.compile()
    return nc


_NC_CACHE = []


def _get_nc():
    if not _NC_CACHE:
        _NC_CACHE.append(build_nc())
    return _NC_CACHE[0]


def kernel(x, emb, Wih_f, Whh_f, b_f, Wih_b, Whh_b, b_b,
           W_h2s, b_h2s, W_s2o, b_s2o):
    from concourse.bass_utils import run_bass_kernel_spmd

    nc = _get_nc()
    in_maps = make_in_maps(x, emb, Wih_f, Whh_f, b_f, Wih_b, Whh_b, b_b,
                           W_h2s, b_h2s, W_s2o, b_s2o)
    last_err = None
    for _attempt in range(3):
        try:
            res = run_bass_kernel_spmd(nc, in_maps, core_ids=list(range(NC)))
            break
        except Exception as e:  # transient NRT device errors: retry
            last_err = e
            import time as _time
            _time.sleep(5)
    else:
        raise last_err
    out = np.concatenate([res.results[c]["out"] for c in range(NC)], axis=0)
    return out.astype(np.float32)


if __name__ == "__main__":
    nc = build_nc()
    print("built + compiled ok")


# revision 53
# speedup vs baseline: 1.0171x; 1.0171x over previous
"""BiLSTM-over-word2vec Trainium2 kernel (8 NeuronCores, SPMD).

Strategy
--------
Data-parallel over the token axis: core c owns tokens [c*1024, (c+1)*1024).
The inherently-sequential LSTM scan is parallelized with chunked warmup:
the LSTM forgets exponentially, so a chunk of L tokens warmed up from zero
state over W extra leading steps reproduces the exact scan state to ~1e-6
by the time real outputs start. Each core runs B = 1024/L chunks per
direction as a batch, so the scan is W+L sequential *batched* steps.

On-chip layout: gates-on-partitions, hidden padded 200->256, gate order
[i, f, o, g~] with g~ pre-scaled x2 so ONE sigmoid covers all gates
(tanh(x) = 2*sigmoid(2x)-1).

Token storage is (j, c)-ordered: the gathered tokens are permuted on the
host so that within each 128-token group, partition p = 8*j + c_local
(j = within-chunk offset, c = chunk). eT/exT are [.., 16 j, 66 c] grids;
every scan step's ex slice is then a CONTIGUOUS 64-column run (v2's
token-ordered exT made the per-step N=512 inject matmul read stride-32B
and cost 4x).

h state lives in contiguous [128, L, 2, B] per-direction buffers indexed
by within-chunk offset, so the recurrent matmul rhs is contiguous.

The embedding table is host-prepped: relu'd, bf16, padded to 384 cols
with a constant bias column (=1) and a flag row at index VOCAB for
out-of-range warmup tokens (-30 on i,f gate columns of Wih freezes
state).
"""

import os
import sys

for _p in ("/opt/trn_rl_repo", "/root/.axon_site/_ro/trn_rl_repo"):
    if os.path.isdir(_p) and _p not in sys.path:
        sys.path.insert(0, _p)

import numpy as np
import ml_dtypes

import concourse.bass as bass
import concourse.mybir as mybir
import concourse.tile as tile
from concourse import bacc
from concourse.bass import IndirectOffsetOnAxis
from concourse.masks import make_identity

BF16 = ml_dtypes.bfloat16

# problem constants (hardcoded per contract)
VOCAB, E, H, EXTRA, OUT, T = 100000, 300, 200, 50, 2, 8192
HP = 256          # padded hidden
G = 4 * HP        # 1024 padded gate rows
NC = 8
SPAN = T // NC    # 1024 tokens per core
L = 16            # chunk length
W = 9             # warmup steps
B = SPAN // L     # 64 chunks per direction per core
STEPS = L + W     # 28
GW = 16           # gather halo (16-aligned so the (j,c) grid is clean)
COLS = SPAN + 2 * GW         # 1056 gathered token slots per core
CPAD = 1152                  # padded to 9 gather groups of 128
NGT = CPAD // 128
CG = COLS // L + 2           # 66 c-columns in the (j,c) grid
EA = 384          # augmented embedding width: 300 emb + bias + flag + pad
F32 = mybir.dt.float32
BF = mybir.dt.bfloat16
SIG = mybir.ActivationFunctionType.Sigmoid
TANH = mybir.ActivationFunctionType.Tanh
RELU = mybir.ActivationFunctionType.Relu
MULT = mybir.AluOpType.mult
ADD = mybir.AluOpType.add
SUB = mybir.AluOpType.subtract

_GATE_SRC = (0, 200, 600, 400)  # gate order [i, f, o, g~] -> orig i,f,g,o offsets


def _reorder_rows(M4h, scale_g=2.0):
    """[4H(orig i,f,g,o), ...] -> [G rows in order i,f,o,g~], g~ scaled."""
    out = np.zeros((G,) + M4h.shape[1:], np.float32)
    for gi, src in enumerate(_GATE_SRC):
        blk = M4h[src:src + H].astype(np.float32)
        if gi == 3:
            blk = blk * scale_g
        out[gi * HP: gi * HP + H] = blk
    return out


def _bf16_hi_lo(a):
    hi = a.astype(BF16)
    lo = (a.astype(np.float32) - hi.astype(np.float32)).astype(BF16)
    return hi, lo


def _prep_weights(Wih_f, Whh_f, b_f, Wih_b, Whh_b, b_b, W_h2s, b_h2s, W_s2o, b_s2o):
    """Host-side weight reordering/padding; returns dict of DRAM input arrays
    shared by all cores (everything except the embedding table and indices)."""
    whh = np.zeros((128, 2, 8, 2, 128), BF16)
    wih = np.zeros((128, 2, 3, G), BF16)
    for d, (Wih_d, Whh_d, b_d) in enumerate(
        ((Wih_f, Whh_f, b_f), (Wih_b, Whh_b, b_b))
    ):
        Whh_r = np.zeros((G, HP), np.float32)
        Whh_r[:, :H] = _reorder_rows(Whh_d)
        whh_bf = Whh_r.astype(BF16)
        for m in range(8):
            for k in range(2):
                # lhsT tile [K=128 (h dims), M=128 (gate rows)]
                whh[:, d, m, k, :] = whh_bf[m * 128:(m + 1) * 128,
                                            k * 128:(k + 1) * 128].T
        Wih_aug = np.zeros((EA, G), np.float32)
        Wih_aug[:E, :] = _reorder_rows(Wih_d).T          # [300, G]
        Wih_aug[E, :] = _reorder_rows(b_d[:, None])[:, 0]  # bias row (col 300=1)
        flagrow = np.zeros(G, np.float32)
        flagrow[:512] = -30.0                             # i,f gate columns
        Wih_aug[E + 1, :] = flagrow                       # flag row (col 301)
        wih[:, d, :, :] = np.stack(
            [Wih_aug[k * 128:(k + 1) * 128].astype(BF16) for k in range(3)], axis=1
        )
    # MLP weights: K space = [hf(256 pad) ; hb(256 pad)] = 512 rows
    W1p = np.zeros((512, 64), np.float32)
    W1p[0:H, :EXTRA] = W_h2s.T[0:H]
    W1p[256:256 + H, :EXTRA] = W_h2s.T[H:2 * H]
    w1hi, w1lo = _bf16_hi_lo(W1p)
    w2s = np.zeros((128, 4, 2, 64), BF16)
    for k in range(4):
        w2s[:, k, 0, :] = w1hi[k * 128:(k + 1) * 128]
        w2s[:, k, 1, :] = w1lo[k * 128:(k + 1) * 128]
    W2p = np.zeros((64, OUT), np.float32)
    W2p[:EXTRA] = W_s2o.T
    w2hi, w2lo = _bf16_hi_lo(W2p)
    ws2o = np.zeros((64, 2, OUT), BF16)
    ws2o[:, 0, :] = w2hi
    ws2o[:, 1, :] = w2lo
    b1 = np.zeros((64, 1), np.float32)
    b1[:EXTRA, 0] = b_h2s.astype(np.float32)
    b2b = np.tile(np.asarray(b_s2o, np.float32).reshape(1, 1, OUT), (128, 4, 1))
    return dict(whh_w=whh, wih_w=wih, w2s_w=w2s, ws2o_w=ws2o, b1=b1, b2b=b2b,
                ident=np.eye(128, dtype=BF16))


def _prep_emb(emb):
    """relu'd bf16 embedding, padded to EA cols with bias col and flag row."""
    ea = np.zeros((VOCAB + 1, EA), BF16)
    ea[:VOCAB, :E] = np.maximum(np.asarray(emb, np.float32), 0.0)
    ea[:, E] = 1.0          # bias column: every gathered token contributes b
    ea[VOCAB, E + 1] = 1.0  # flag column set only on the invalid-token row
    return ea


def _prep_core_inputs(x, core):
    """Per-core token indices [128, NGT], (j,c)-permuted within each group:
    gathered slot s = 128*g + 16*c_local + j lands on partition 8*j + c_local,
    so the PE transpose emits (j, c)-ordered columns. Invalid slots -> VOCAB."""
    base = core * SPAN
    toks = np.arange(base - GW, base + SPAN + GW, dtype=np.int64)
    invalid = (toks < 0) | (toks >= T)
    tokc = np.clip(toks, 0, T - 1)
    xi = x[tokc].astype(np.int64)
    xi = np.where(xi < 0, 0, xi)          # masked tokens never occur (randint)
    xi = np.where(invalid, VOCAB, xi)
    idx = np.zeros(CPAD, np.int32)
    idx[:COLS] = xi.astype(np.int32)
    idxg = idx.reshape(NGT, 8, L)          # [g, c_local, j]
    idxp = np.ascontiguousarray(idxg.transpose(2, 1, 0)).reshape(128, NGT)
    return dict(xidx=idxp.copy())


def make_in_maps(x, emb, Wih_f, Whh_f, b_f, Wih_b, Whh_b, b_b,
                 W_h2s, b_h2s, W_s2o, b_s2o):
    shared = _prep_weights(Wih_f, Whh_f, b_f, Wih_b, Whh_b, b_b,
                           W_h2s, b_h2s, W_s2o, b_s2o)
    shared["emb"] = _prep_emb(emb)
    x = np.asarray(x)
    in_maps = []
    for core in range(NC):
        m = dict(shared)
        m.update(_prep_core_inputs(x, core))
        in_maps.append(m)
    return in_maps


def build_nc():
    nc = bacc.Bacc("TRN2", target_bir_lowering=False, debug=False, num_devices=NC)

    emb_t = nc.dram_tensor("emb", [VOCAB + 1, EA], BF, kind="ExternalInput").ap()
    xidx_t = nc.dram_tensor("xidx", [128, NGT], mybir.dt.int32, kind="ExternalInput").ap()
    ident_t = nc.dram_tensor("ident", [128, 128], BF, kind="ExternalInput").ap()
    whh_t = nc.dram_tensor("whh_w", [128, 2, 8, 2, 128], BF, kind="ExternalInput").ap()
    wih_t = nc.dram_tensor("wih_w", [128, 2, 3, G], BF, kind="ExternalInput").ap()
    w2s_t = nc.dram_tensor("w2s_w", [128, 4, 2, 64], BF, kind="ExternalInput").ap()
    ws2o_t = nc.dram_tensor("ws2o_w", [64, 2, OUT], BF, kind="ExternalInput").ap()
    b1_t = nc.dram_tensor("b1", [64, 1], F32, kind="ExternalInput").ap()
    b2b_t = nc.dram_tensor("b2b", [128, 4, OUT], F32, kind="ExternalInput").ap()
    out_t = nc.dram_tensor("out", [SPAN, OUT], F32, kind="ExternalOutput").ap()

    with tile.TileContext(nc) as tc:
        with tc.tile_pool(name="const", bufs=1) as const:
            # DMA priority: idx+ident gate the gathers/spin, wih gates the exT
            # matmuls (and must beat the gather descriptor flood); whh is not
            # needed until scan step 1, so it goes last.
            idx_sb = const.tile([128, NGT], mybir.dt.int32, tag="idx")
            nc.sync.dma_start(out=idx_sb[:], in_=xidx_t)
            identb = const.tile([128, 128], BF, tag="identb")
            nc.sync.dma_start(out=identb[:], in_=ident_t)
            wih_sb = const.tile([128, 2, 3, G], BF, tag="wih")
            nc.sync.dma_start(out=wih_sb[:], in_=wih_t)
            w2s_sb = const.tile([128, 4, 2, 64], BF, tag="w2s")
            nc.sync.dma_start(out=w2s_sb[:], in_=w2s_t)
            ws2o_sb = const.tile([64, 2, OUT], BF, tag="ws2o")
            nc.sync.dma_start(out=ws2o_sb[:], in_=ws2o_t)
            b1_sb = const.tile([64, 1], F32, tag="b1")
            nc.sync.dma_start(out=b1_sb[:], in_=b1_t)
            b2b_sb = const.tile([128, 4, OUT], F32, tag="b2b")
            nc.sync.dma_start(out=b2b_sb[:], in_=b2b_t)
            whh_sb = const.tile([128, 2, 8, 2, 128], BF, tag="whh")
            nc.sync.dma_start(out=whh_sb[:], in_=whh_t)
            ident = identb

            # (j, c) grids
            eT = [const.tile([128, L, CG], BF, tag=f"eT{k}", name=f"eT{k}")
                  for k in range(3)]
            exT = [const.tile([128, 8, L, CG], BF, tag=f"exT{d}", name=f"exT{d}")
                   for d in range(2)]
            # h state: [128 hdim-parts, L within-chunk slot, 2 hdim-halves, B]
            hbuf = [const.tile([128, L, 2, B], BF, tag=f"hb{d}", name=f"hb{d}")
                    for d in range(2)]
            # staged MLP outputs: rows 0:64 slot s = token (c, j=s);
            # rows 64:128 slot s = token (c, j=8+s)
            obuf = const.tile([128, 8, OUT], F32, tag="obuf")

            # scan state pools (outer: step 0 is emitted inside the gather
            # block so it isn't FIFO-blocked behind direction 1's exT copies)
            scan_stack = tc.tile_pool(name="act", bufs=3)
            ap_ = scan_stack.__enter__()
            cstate_stack = tc.tile_pool(name="cstate", bufs=3)
            cp = cstate_stack.__enter__()
            scr_stack = tc.tile_pool(name="scr", bufs=3)
            scr = scr_stack.__enter__()

            def ex_slice(d, sp):
                q = (sp + GW - W) if d == 0 else (GW + L + W - 1 - sp)
                jj, cl = q % L, q // L
                return exT[d][:, :, jj, cl:cl + B]

            c_prev = [None, None]
            h_prev = [None, None]

            def hdst_for(d, sp):
                j = (sp - W) if d == 0 else (L + W - 1 - sp)
                if sp >= W:
                    return hbuf[d][:, j, :, :]
                hw = scr.tile([128, 2, B], BF, tag=f"hw{d}", name=f"hw{d}")
                return hw[:]

            def sp0_dir(d):
                # step 0, one direction: no h yet -> gates = sigmoid(ex)
                a0 = ap_.tile([128, 8, B], F32, tag=f"a{d}", name=f"a{d}")
                nc.scalar.activation(a0[:], ex_slice(d, 0), SIG)
                t0 = scr.tile([128, 2, B], F32, tag=f"t{d}", name=f"t{d}")
                cn0 = cp.tile([128, 2, B], F32, tag=f"c{d}", name=f"c{d}")
                tc0 = scr.tile([128, 2, B], F32, tag=f"tc{d}", name=f"tc{d}")
                nc.vector.tensor_tensor(
                    out=t0[:], in0=a0[:, 0:2, :], in1=a0[:, 6:8, :], op=MULT)
                nc.vector.scalar_tensor_tensor(
                    out=cn0[:], in0=t0[:], scalar=2.0,
                    in1=a0[:, 0:2, :], op0=MULT, op1=SUB)
                nc.scalar.activation(tc0[:], cn0[:], TANH)
                hd = hdst_for(d, 0)
                nc.gpsimd.tensor_tensor(
                    out=hd, in0=a0[:, 4:6, :], in1=tc0[:], op=MULT)
                h_prev[d] = hd
                c_prev[d] = cn0[:]

            # ---- gather + transpose + exT, interleaved so exT matmuls start
            # after only the gather groups they need (Tensor queue is FIFO) ----
            cslabs = [(0, 26), (26, 26), (52, 14)]
            with (
                tc.tile_pool(name="gath", bufs=1) as gp,
                tc.tile_pool(name="gpsum", bufs=4, space="PSUM") as gps,
                tc.tile_pool(name="expsum", bufs=3, space="PSUM") as exps,
            ):
                # gathers first: nothing on gpsimd before them, so the
                # indirect-DMA issues (~1.4us each, serial) start immediately
                ets = []
                for g in range(NGT):
                    et = gp.tile([128, EA], BF, tag=f"ge{g}", name=f"ge{g}")
                    nc.gpsimd.indirect_dma_start(
                        out=et[:],
                        out_offset=None,
                        in_=emb_t,
                        in_offset=IndirectOffsetOnAxis(ap=idx_sb[:, g:g + 1], axis=0),
                    )
                    ets.append(et)

                # PE spin: lift the HAM clock gate before the transposes
                with tc.tile_pool(name="warm", bufs=1, space="PSUM") as wp:
                    wps = wp.tile([128, 128], F32, tag="warm")
                    for _ in range(16):
                        nc.tensor.matmul(out=wps[:], lhsT=ident[:], rhs=ident[:],
                                         start=True, stop=True)

                def emit_transposes(glist):
                    for g in glist:
                        cw = 8 if g < NGT - 1 else 2   # last group: 2 real c's
                        for kc in range(3):
                            pt = gps.tile([128, L, 8], BF, tag="tr", name="pt")
                            nc.tensor.transpose(
                                out=pt[:], in_=ets[g][:, kc * 128:(kc + 1) * 128],
                                identity=ident[:],
                            )
                            if (g + kc) % 2 == 0:
                                nc.vector.tensor_copy(
                                    out=eT[kc][:, :, 8 * g:8 * g + cw],
                                    in_=pt[:, :, :cw],
                                )
                            else:
                                nc.scalar.copy(
                                    out=eT[kc][:, :, 8 * g:8 * g + cw],
                                    in_=pt[:, :, :cw],
                                )

                def emit_exslab(d, si):
                    c0, cn = cslabs[si]
                    for m in range(8):
                        ps = exps.tile([128, L, 26], F32, tag="exps",
                                       name="exps")
                        if cn == 26:
                            mm_out = cp_in = ps[:]
                        else:  # pack short slab contiguously in PSUM
                            flat = ps[:].rearrange("p j c -> p (j c)")
                            mm_out = flat[:, :L * cn]
                            cp_in = mm_out.rearrange("p (j c) -> p j c",
                                                     j=L, c=cn)
                        for k in range(3):
                            nc.tensor.matmul(
                                out=mm_out,
                                lhsT=wih_sb[:, d, k, m * 128:(m + 1) * 128],
                                rhs=eT[k][:, :, c0:c0 + cn],
                                start=(k == 0),
                                stop=(k == 2),
                            )
                        if (d + m + si) % 2 == 0:
                            nc.vector.tensor_copy(
                                out=exT[d][:, m, :, c0:c0 + cn], in_=cp_in)
                        else:
                            nc.scalar.copy(
                                out=exT[d][:, m, :, c0:c0 + cn], in_=cp_in)

                emit_transposes([0, 1, 2, 3])
                emit_exslab(0, 0)
                emit_transposes([4, 5, 6])
                emit_exslab(1, 0)
                emit_transposes([7, 8])
                emit_exslab(0, 1)
                emit_exslab(1, 1)
                emit_exslab(0, 2)
                sp0_dir(0)          # d0 step 0 runs while d1's last slab computes
                emit_exslab(1, 2)
                sp0_dir(1)

            # ---- the scan, steps 1.., with the MLP head fused in as
            # per-j-pair passes (pair p = offsets (7-p, 8+p) is complete
            # after step 20+p; its MLP matmuls replace dummy drip) ----
            out_j = out_t.rearrange("(c j) o -> j c o", j=L)
            with (
                tc.tile_pool(name="pg", bufs=2, space="PSUM") as pgp,
                tc.tile_pool(name="dummy", bufs=1, space="PSUM") as dpp,
                tc.tile_pool(name="mlp", bufs=1, space="PSUM") as mpp,
            ):
                dps = dpp.tile([128, 128], F32, tag="dummy")

                def emit_mlp_pair(p):
                    jlo, jhi = 7 - p, 8 + p
                    ps = mpp.tile([64, 128], F32, tag="mps", name="mps")
                    mmi = 0
                    for d in range(2):
                        for k in range(2):
                            for hl in range(2):
                                nc.tensor.matmul(
                                    out=ps[:],
                                    lhsT=w2s_sb[:, d * 2 + k, hl, :],
                                    rhs=hbuf[d][:, jlo:jhi + 1:(jhi - jlo), k, :],
                                    start=(mmi == 0), stop=(mmi == 7),
                                )
                                mmi += 1
                    s32 = scr.tile([64, 128], F32, tag="ms32", name="ms32")
                    nc.scalar.activation(s32[:], ps[:], RELU, bias=b1_sb[:])
                    shi = scr.tile([64, 128], BF, tag="mshi", name="mshi")
                    nc.vector.tensor_copy(out=shi[:], in_=s32[:])
                    slo = scr.tile([64, 128], BF, tag="mslo", name="mslo")
                    nc.vector.tensor_tensor(
                        out=slo[:], in0=s32[:], in1=shi[:], op=SUB)
                    po = mpp.tile([128, OUT], F32, tag="mpo", name="mpo")
                    for oi, (shl, whl) in enumerate(((shi, 0), (shi, 1), (slo, 0))):
                        nc.tensor.matmul(
                            out=po[:], lhsT=shl[:], rhs=ws2o_sb[:, whl, :],
                            start=(oi == 0), stop=(oi == 2),
                        )
                    nc.vector.tensor_tensor(
                        out=obuf[0:64, jlo, :], in0=po[0:64, :],
                        in1=b2b_sb[0:64, 0, :], op=ADD)
                    nc.vector.tensor_tensor(
                        out=obuf[64:128, jhi - 8, :], in0=po[64:128, :],
                        in1=b2b_sb[64:128, 0, :], op=ADD)

                for sp in range(1, STEPS):
                    a = [ap_.tile([128, 8, B], F32, tag=f"a{d}", name=f"a{d}")
                         for d in range(2)]
                    pss = [pgp.tile([128, 8, B], F32, tag=f"pg{d}", name=f"pg{d}")
                           for d in range(2)]
                    # ex inject: one contiguous N=512 identity matmul per dir
                    for d in range(2):
                        nc.tensor.matmul(
                            out=pss[d][:], lhsT=ident[:], rhs=ex_slice(d, sp),
                            start=True, stop=False,
                        )
                    # dummy drip right before the h-dependent matmuls: keeps
                    # the PE non-idle while the previous step's chain finishes
                    # (the HAM clock gate re-throttles on any idle window);
                    # late steps get real MLP matmuls instead
                    for _ in range(36 if sp < W + 9 else 12):
                        nc.tensor.matmul(out=dps[:, :64], lhsT=ident[:],
                                         rhs=ident[:, :64], start=True, stop=True)
                    for m in range(8):
                        for k in range(2):
                            nc.tensor.matmul(
                                out=pss[0][:, m, :],
                                lhsT=whh_sb[:, 0, m, k, :],
                                rhs=h_prev[0][:, k, :],
                                start=False, stop=(k == 1),
                            )
                    nc.scalar.activation(a[0][:], pss[0][:], SIG)
                    for m in range(8):
                        for k in range(2):
                            nc.tensor.matmul(
                                out=pss[1][:, m, :],
                                lhsT=whh_sb[:, 1, m, k, :],
                                rhs=h_prev[1][:, k, :],
                                start=False, stop=(k == 1),
                            )
                    if sp >= W + 9:
                        emit_mlp_pair(sp - (W + 9))
                    # gate math; engine queues ordered by operand readiness:
                    #   ACT: [sig0, sig1, tanh0, tanh1]
                    #   DVE: [t0, u0, c0, h0, t1, u1, c1, h1]
                    #   GpSimd: [r0, r1]
                    t = [scr.tile([128, 2, B], F32, tag=f"t{d}", name=f"t{d}")
                         for d in range(2)]
                    u = [scr.tile([128, 2, B], F32, tag=f"u{d}", name=f"u{d}")
                         for d in range(2)]
                    r = [scr.tile([128, 2, B], F32, tag=f"r{d}", name=f"r{d}")
                         for d in range(2)]
                    cnew = [cp.tile([128, 2, B], F32, tag=f"c{d}", name=f"c{d}")
                            for d in range(2)]
                    tct = [scr.tile([128, 2, B], F32, tag=f"tc{d}", name=f"tc{d}")
                          for d in range(2)]
                    hd = [None, None]

                    nc.vector.tensor_tensor(
                        out=t[0][:], in0=a[0][:, 0:2, :], in1=a[0][:, 6:8, :], op=MULT)
                    nc.vector.scalar_tensor_tensor(
                        out=u[0][:], in0=t[0][:], scalar=2.0,
                        in1=a[0][:, 0:2, :], op0=MULT, op1=SUB)
                    nc.scalar.activation(a[1][:], pss[1][:], SIG)
                    nc.gpsimd.tensor_tensor(
                        out=r[0][:], in0=a[0][:, 2:4, :], in1=c_prev[0], op=MULT)
                    nc.vector.tensor_tensor(
                        out=cnew[0][:], in0=r[0][:], in1=u[0][:], op=ADD)
                    nc.scalar.activation(tct[0][:], cnew[0][:], TANH)
                    hd[0] = hdst_for(0, sp)
                    nc.vector.tensor_tensor(
                        out=hd[0], in0=a[0][:, 4:6, :], in1=tct[0][:], op=MULT)
                    nc.gpsimd.tensor_tensor(
                        out=r[1][:], in0=a[1][:, 2:4, :], in1=c_prev[1], op=MULT)
                    nc.vector.tensor_tensor(
                        out=t[1][:], in0=a[1][:, 0:2, :], in1=a[1][:, 6:8, :], op=MULT)
                    nc.vector.scalar_tensor_tensor(
                        out=u[1][:], in0=t[1][:], scalar=2.0,
                        in1=a[1][:, 0:2, :], op0=MULT, op1=SUB)
                    nc.vector.tensor_tensor(
                        out=cnew[1][:], in0=r[1][:], in1=u[1][:], op=ADD)
                    nc.scalar.activation(tct[1][:], cnew[1][:], TANH)
                    hd[1] = hdst_for(1, sp)
                    nc.vector.tensor_tensor(
                        out=hd[1], in0=a[1][:, 4:6, :], in1=tct[1][:], op=MULT)
                    for d in range(2):
                        h_prev[d] = hd[d]
                        c_prev[d] = cnew[d][:]

                emit_mlp_pair(7)   # last pair (j = 0, 15)
                # two contiguous output DMAs (row t = 16c + j2*8 + jj)
                out_r2 = out_t.rearrange("(c j2 jj) o -> j2 c jj o",
                                         j2=2, jj=8)
                nc.sync.dma_start(out=out_r2[0], in_=obuf[0:64, :, :])
                nc.sync.dma_start(out=out_r2[1], in_=obuf[64:128, :, :])

            scr_stack.__exit__(None, None, None)
            cstate_stack.__exit__(None, None, None)
            scan_stack.__exit__(None, None, None)

    nc.compile()
    return nc


_NC_CACHE = []


def _get_nc():
    if not _NC_CACHE:
        _NC_CACHE.append(build_nc())
    return _NC_CACHE[0]


def kernel(x, emb, Wih_f, Whh_f, b_f, Wih_b, Whh_b, b_b,
           W_h2s, b_h2s, W_s2o, b_s2o):
    from concourse.bass_utils import run_bass_kernel_spmd

    nc = _get_nc()
    in_maps = make_in_maps(x, emb, Wih_f, Whh_f, b_f, Wih_b, Whh_b, b_b,
                           W_h2s, b_h2s, W_s2o, b_s2o)
    last_err = None
    for _attempt in range(3):
        try:
            res = run_bass_kernel_spmd(nc, in_maps, core_ids=list(range(NC)))
            break
        except Exception as e:  # transient NRT device errors: retry
            last_err = e
            import time as _time
            _time.sleep(5)
    else:
        raise last_err
    out = np.concatenate([res.results[c]["out"] for c in range(NC)], axis=0)
    return out.astype(np.float32)


if __name__ == "__main__":
    nc = build_nc()
    print("built + compiled ok")
